# revision 1
# baseline (speedup 1.0000x reference)
"""MinkowskiBroadcast gather kernel for Trainium2: out[i] = x_glob[batch_idx[i]].

Full inputs in, full output out.  Internally data-parallel over points across
8 NeuronCores (batch_idx and output rows sharded; the [32,128] feature table
replicated), per the sharding hint.

Per-core device strategy (SPMD, one NEFF):
  - x_glob is split on-device into bf16 hi/lo (x = hi + lo, ~2^-16 rel
    precision; PSUM accumulates in fp32) and packed [128,128] bf16:
    partitions [0:32]=hi [32:64]=lo [64:96]=hi [96:128]=lo.
  - indices arrive host-permuted + bf16-cast as idxp[s, q, n] with
    n = 4*m + r encoding point t = 2048*s + 16*m + 4*q + r.
  - per super-tile s (2048 points):
      broadcast matmul: psum_b[p, n] = idx[q-block(p//64), n] (selector lhsT)
      DVE is_equal vs per-partition iota (p % 32) -> one-hot bf16 in SBUF
      16 gather matmuls: K=64 (hi;lo stacked) x M=128 points x N=128 chans
      PSUM -> SBUF copies split DVE/ACT (192/832 cols per 1024-col half,
      balancing the two engines' measured copy rates)
      one 1 MiB DMA out; per partition m, output rows [16m .. 16m+15] are
      contiguous in HBM -> 8 KiB descriptors; the final super-tile's DMA is
      clipped to the 125,000 valid rows.
"""

from contextlib import ExitStack

import numpy as np
import ml_dtypes

import concourse.bass as bass
import concourse.mybir as mybir
import concourse.tile as tile
from concourse.bass_utils import run_bass_kernel_spmd

F32 = mybir.dt.float32
BF16 = mybir.dt.bfloat16

N_POINTS = 1_000_000
N_BATCH = 32
C = 128
N_CORES = 8
N_SHARD = N_POINTS // N_CORES          # 125_000
SUPER = 2048                           # points per super-tile
N_SUPERS = -(-N_SHARD // SUPER)        # 62
N_PAD = N_SUPERS * SUPER               # 126_976


def _split_waits(nc: bass.Bass, max_waits: int = 1) -> None:
    """This walrus build rejects instructions carrying more than one sync
    wait ("Too many sync wait commands").  Hoist extra waits onto dedicated
    no-op instructions inserted just before, on the same engine."""
    n = 0
    for f in nc.m.functions:
        for bb in f.blocks:
            out = []
            changed = False
            for inst in bb.instructions:
                si = inst.sync_info
                waits = list(si.on_wait) if si and si.on_wait else []
                if len(waits) > max_waits:
                    for w in waits[:-max_waits]:
                        nop = mybir.InstNoOp(
                            name=f"WSPLIT-{n}", engine=inst.engine, ins=[], outs=[],
                            sync_info=mybir.SyncInfo(on_wait=[w], on_update=[]),
                        )
                        n += 1
                        out.append(nop)
                    si.on_wait = waits[-max_waits:]
                    changed = True
                out.append(inst)
            if changed:
                bb.instructions = out


def build_gather_bass(n_supers: int = N_SUPERS, dve_cols: int = 192,
                      split: bool = True, idx_batch: int = 8,
                      idx_ring: str = "act", bufs: dict | None = None,
                      repeat: int = 1, n_valid: int | None = None,
                      iseq_split: bool = False, out_ring: str = "sp",
                      out_halves: int = 1, _skip: tuple = ()) -> bass.Bass:
    """Build the per-core gather program for n_supers super-tiles of 2048 pts.

    dve_cols: how many of each 1024-col half-super copy DVE takes (ACT rest).
    idx_batch: supers per index-load DMA.
    """
    S = n_supers
    IB = idx_batch
    B = {"idx": 2, "oh": 3, "osb": 4, "po": 3, "pb": 1}
    B.update(bufs or {})
    if n_valid is None:
        n_valid = S * SUPER
    assert S * SUPER - SUPER < n_valid <= S * SUPER
    nc = bass.Bass(trn_type="TRN2")

    x_glob = nc.dram_tensor("x_glob", [N_BATCH, C], F32, kind="ExternalInput")
    idxp = nc.dram_tensor("idxp", [S, 4, 512], BF16, kind="ExternalInput")
    sel = nc.dram_tensor("sel", [2, 128], BF16, kind="ExternalInput")
    iota = nc.dram_tensor("iota", [128, 1], F32, kind="ExternalInput")
    out = nc.dram_tensor("out", [n_valid, C], F32, kind="ExternalOutput")
    scratch = (nc.dram_tensor("scratch", [S * SUPER, C], F32)
               if repeat > 1 else None)

    def super_dma(nc, osb, s, to_scratch):
        """DMA one super-tile's osb [128, 2048] to rows [2048s, 2048(s+1));
        clip the final super to n_valid rows.  Per partition m the 16 rows
        [16m .. 16m+15] are one contiguous 8KB run."""
        tgt = scratch if to_scratch else out
        lo = s * SUPER
        hi = min(lo + SUPER, S * SUPER if to_scratch else n_valid)
        mfull = (hi - lo) // 16
        if mfull:
            ap = tgt[lo:lo + mfull * 16].rearrange("(m x) c -> m (x c)", m=mfull)
            if out_ring == "split":
                nc.sync.dma_start(out=ap[:, 0:1024], in_=osb[0:mfull, 0:1024])
                nc.scalar.dma_start(out=ap[:, 1024:2048], in_=osb[0:mfull, 1024:2048])
            elif out_ring == "alt":
                eng = nc.sync if s % 2 == 0 else nc.scalar
                eng.dma_start(out=ap, in_=osb[0:mfull, 0:16 * C])
            elif out_halves > 1 and mfull == 128:
                w = 2048 // out_halves
                for h in range(out_halves):
                    nc.sync.dma_start(out=ap[:, h * w:(h + 1) * w],
                                      in_=osb[:, h * w:(h + 1) * w])
            else:
                nc.sync.dma_start(out=ap, in_=osb[0:mfull, 0:16 * C])
        rem = (hi - lo) % 16
        if rem:
            ap = tgt[lo + mfull * 16:hi].rearrange("(m x) c -> m (x c)", m=1)
            nc.sync.dma_start(out=ap, in_=osb[mfull:mfull + 1, 0:rem * C])

    with tile.TileContext(nc) as tc, ExitStack() as ctx:
        singles = ctx.enter_context(tc.tile_pool(name="singles", bufs=1))
        idxpool = ctx.enter_context(tc.tile_pool(name="idx", bufs=B["idx"]))
        ohpool = ctx.enter_context(tc.tile_pool(name="oh", bufs=B["oh"]))
        osbpool = ctx.enter_context(tc.tile_pool(name="osb", bufs=B["osb"]))
        pbpool = ctx.enter_context(tc.tile_pool(name="pb", bufs=B["pb"], space="PSUM"))
        popool = ctx.enter_context(tc.tile_pool(name="po", bufs=B["po"], space="PSUM"))

        # ---- constants / x_glob hi-lo packing ----
        xg = singles.tile([N_BATCH, C], F32)
        nc.scalar.dma_start(out=xg[:], in_=x_glob[:])
        selsb = singles.tile([2, 128], BF16)
        nc.scalar.dma_start(out=selsb[:], in_=sel[:])
        iotasb = singles.tile([128, 1], F32)
        nc.scalar.dma_start(out=iotasb[:], in_=iota[:])

        hi_bf = singles.tile([N_BATCH, C], BF16)
        hi_f32 = singles.tile([N_BATCH, C], F32)
        lo_bf = singles.tile([N_BATCH, C], BF16)
        nc.vector.tensor_copy(out=hi_bf[:], in_=xg[:])
        nc.vector.tensor_copy(out=hi_f32[:], in_=hi_bf[:])
        nc.vector.tensor_sub(out=lo_bf[:], in0=xg[:], in1=hi_f32[:])

        xpack = singles.tile([128, C], BF16)
        nc.sync.dma_start(out=xpack[0:32, :], in_=hi_bf[:])
        nc.sync.dma_start(out=xpack[32:64, :], in_=lo_bf[:])
        nc.sync.dma_start(out=xpack[64:96, :], in_=hi_bf[:])
        nc.sync.dma_start(out=xpack[96:128, :], in_=lo_bf[:])

        # ---- main loop over super-tiles ----
        idxt = None
        for rep in range(repeat):
          to_scratch = rep != repeat - 1
          for s in range(S):
            if s % IB == 0:
                L = min(IB, S - s)
                idxt = idxpool.tile([2, L, 2, 512], BF16, tag="idxt")
                # [k, s', qh, n] with q = 2*qh + k
                batch_ap = idxp[s:s + L].rearrange("s (qh k) n -> k s qh n", k=2)
                (nc.scalar if idx_ring == "act" else nc.sync).dma_start(out=idxt[:], in_=batch_ap)
            sv = s % IB

            # broadcast idx to 32-partition blocks: pb[64*qh+64Q+b, n] holds
            # idx of q-block q=2*qh+Q (duplicated over b%32 pairs)
            pb = pbpool.tile([128, 1024], F32, tag="pb")
            for qh in range(2):
                nc.tensor.matmul(pb[:, qh * 512:(qh + 1) * 512], selsb[:],
                                 idxt[:, sv, qh, :], start=True, stop=True)
            oh = ohpool.tile([128, 1024], BF16, tag="oh")
            if iseq_split:
                for qh in range(2):
                    nc.vector.tensor_scalar(
                        out=oh[:, qh * 512:(qh + 1) * 512],
                        in0=pb[:, qh * 512:(qh + 1) * 512],
                        scalar1=iotasb[:], scalar2=None,
                        op0=mybir.AluOpType.is_equal,
                    )
            else:
                nc.vector.tensor_scalar(
                    out=oh[:], in0=pb[:], scalar1=iotasb[:], scalar2=None,
                    op0=mybir.AluOpType.is_equal,
                )

            osb = osbpool.tile([128, 2048], F32, tag="osb")
            for qh in range(2):
                ohr = oh[:, qh * 512:(qh + 1) * 512].rearrange("p (m r) -> p r m", r=4)
                po = popool.tile([128, 1024], F32, tag="po")
                for Q in range(2):
                    for r in range(4):
                        nc.tensor.matmul(
                            po[:, Q * 512 + r * 128:Q * 512 + (r + 1) * 128],
                            ohr[64 * Q:64 * (Q + 1), r, :],
                            xpack[64 * Q:64 * (Q + 1), :],
                            start=True, stop=True,
                        )
                c0 = qh * 1024
                if "copies" not in _skip:
                    if dve_cols > 0:
                        nc.vector.tensor_copy(out=osb[:, c0:c0 + dve_cols],
                                              in_=po[:, 0:dve_cols])
                    if dve_cols < 1024:
                        nc.scalar.copy(out=osb[:, c0 + dve_cols:c0 + 1024],
                                       in_=po[:, dve_cols:1024])

            if "outdma" not in _skip:
                super_dma(nc, osb, s, to_scratch)

    if split:
        _split_waits(nc)
    return nc


def host_prep_idx(batch_idx_shard: np.ndarray, n_supers: int = N_SUPERS) -> np.ndarray:
    """Pad + permute + cast one core's index shard for the device layout:
    idxp[s, q, m*4+r] = idx[2048 s + 16 m + 4 q + r]."""
    npad = n_supers * SUPER
    idx = np.zeros([npad], np.int32)
    idx[: batch_idx_shard.shape[0]] = batch_idx_shard
    return (
        idx.reshape(n_supers, 128, 4, 4)
        .transpose(0, 2, 1, 3)
        .reshape(n_supers, 4, 512)
        .astype(ml_dtypes.bfloat16)
    )


def host_consts():
    sel = (np.arange(128)[None, :] // 64 == np.arange(2)[:, None]).astype(
        ml_dtypes.bfloat16
    )
    iota = (np.arange(128) % 32).astype(np.float32).reshape(128, 1)
    return sel, iota


_NC_CACHE: dict = {}


def _get_nc() -> bass.Bass:
    if "nc" not in _NC_CACHE:
        _NC_CACHE["nc"] = build_gather_bass(n_valid=N_SHARD)
    return _NC_CACHE["nc"]


def make_in_maps(x_glob: np.ndarray, batch_idx: np.ndarray) -> list:
    x = np.ascontiguousarray(np.asarray(x_glob, dtype=np.float32))
    idx = np.asarray(batch_idx, dtype=np.int32)
    sel, iota = host_consts()
    return [
        {
            "x_glob": x,
            "idxp": host_prep_idx(idx[i * N_SHARD:(i + 1) * N_SHARD]),
            "sel": sel,
            "iota": iota,
        }
        for i in range(N_CORES)
    ]


def kernel(x_glob: np.ndarray, batch_idx: np.ndarray) -> np.ndarray:
    res = run_bass_kernel_spmd(
        _get_nc(), make_in_maps(x_glob, batch_idx), core_ids=list(range(N_CORES))
    )
    return np.concatenate(
        [res.results[i]["out"][:N_SHARD] for i in range(N_CORES)], axis=0
    )

